# revision 12
# baseline (speedup 1.0000x reference)
"""MoE routing kernel for 8 Trainium2 NeuronCores.

Problem: B=65536 tokens, shared Linear(512->256)+ReLU, then per-token expert
MLP Linear(256->100)+ReLU -> Linear(100->1), expert chosen by idx in [0,16).

Strategy (expert-parallel, host-side routing):
  - Host sorts tokens by expert. Experts 2c and 2c+1 go to core c, each in a
    fixed-capacity slot of C tokens (C = max expert count rounded up to 128),
    padded with token 0 (padding outputs are computed then discarded).
  - Host pre-transposes x to [512, TOK] bf16 per core so the contraction dim
    (IN_DIM) lands on SBUF partitions: all three GEMMs then chain on-chip with
    no transposes (layer-1 out [hid, tok] feeds layer-2, which feeds layer-3).
  - Device: per group of <=512 tokens: one DMA of x columns, 8 accumulating
    matmuls (512-dim contraction, 2 hid chunks) + bias/ReLU, 2 matmuls for
    expert FC1 + bias/ReLU, 1 matmul for FC2 (bias folded in via a ones-row).
  - All weights/biases ride in TWO packed contiguous bf16 DMAs on the ACT
    HWDGE ring; the SP ring carries only the x stream, so the first x group
    lands as early as possible.
  - Layer-3 outputs for 8 consecutive groups accumulate into one [8, 512]
    PSUM tile (per-group stationary is [128, 8] with a single nonzero column),
    so the store path is 3 copies + 3 multi-partition DMAs instead of a slow
    single-partition tail.
"""

import math
import os
import sys

import numpy as np

for _p in ("/opt/trn_rl_repo", "/opt/pypackages"):
    if _p not in sys.path and os.path.isdir(_p):
        sys.path.append(_p)

import ml_dtypes

BF16 = ml_dtypes.bfloat16

B, IN_DIM, HID, EXP_HID, OUT_DIM, N_EXP = 65536, 512, 256, 100, 1, 16
N_CORES = 8
GROUP = 512  # tokens per matmul group (= PSUM bank free-dim in fp32)
SB = 8  # groups per output super-block

# packed const layouts (bf16 columns per partition); biases are f32 stored
# as raw bytes (2 bf16 cols per value) and bitcast on device.
# constA is hc-major and split into two DMAs so the first L1 matmuls only
# gate on the first half: [ws hc0 512 | bs 4 | ws hc1 512]
CA_BS = 512
CA_COLS = 1028
CB_W1, CB_B1, CB_W2 = 0, 512, 516  # constB: w1 [2*2*128], b1 [2 f32], w2sel


def _ca_col(hc, kc):
    return hc * 516 + kc * 128

_PROGRAM_CACHE = {}


def _make_groups(C):
    """Execution-ordered blocks: (slot, off, n).

    Remainder (short) blocks run last: they form the final output super-block,
    so the last L3 -> copy -> store chain is short.
    """
    full, rem = [], []
    for slot in range(2):
        off = 0
        while off < C:
            n = min(GROUP, C - off)
            (full if n == GROUP else rem).append((slot, off, n))
            off += n
    return full + rem


def _build_program(C: int):
    """Build (and cache) the Bass program for per-expert-slot capacity C."""
    import concourse.bass as bass
    import concourse.mybir as mybir
    import concourse.tile as tile
    from concourse import bacc

    TOK = 2 * C
    f32 = mybir.dt.float32
    bf16 = mybir.dt.bfloat16
    AF = mybir.ActivationFunctionType
    ALU = mybir.AluOpType

    groups = _make_groups(C)
    n_groups = len(groups)
    n_sb = (n_groups + SB - 1) // SB
    cb_cols = CB_W2 + 8 * n_groups

    nc = bacc.Bacc("TRN2", target_bir_lowering=False, debug=False)

    # xg[i] holds the i-th executed group's x block: for full groups
    # xg[i, p, kc*512 + t] = x[tok, kc*128 + p]; remainder groups are packed
    # compactly as xg[i, p, kc*n + t] (contiguous HBM reads).
    xg_d = nc.dram_tensor(
        "xg", [n_groups, 128, 4 * GROUP], bf16, kind="ExternalInput"
    ).ap()
    ca_d = nc.dram_tensor("ca", [128, CA_COLS], bf16, kind="ExternalInput").ap()
    cb_d = nc.dram_tensor("cb", [128, cb_cols], bf16, kind="ExternalInput").ap()
    out_d = nc.dram_tensor("out", [SB, n_sb, GROUP], f32, kind="ExternalOutput").ap()

    with tile.TileContext(nc) as tc:
        with (
            tc.tile_pool(name="const", bufs=1) as const,
            tc.tile_pool(name="xp", bufs=8) as xp,
            tc.tile_pool(name="hp", bufs=3) as hp,
            tc.tile_pool(name="h1p", bufs=3) as h1p,
            tc.tile_pool(name="ps1", bufs=4, space="PSUM") as ps1,
            tc.tile_pool(name="ps2", bufs=2, space="PSUM") as ps2,
            tc.tile_pool(name="ps3", bufs=2, space="PSUM") as ps3,
        ):
            ca_sb = const.tile([128, CA_COLS], bf16)
            cb_sb = const.tile([128, cb_cols], bf16)
            o2 = const.tile([SB, n_sb, GROUP], f32)
            bs_f32 = ca_sb[:, CA_BS : CA_BS + 4].bitcast(f32)  # [128, 2]
            b1_f32 = cb_sb[:, CB_B1 : CB_B1 + 4].bitcast(f32)  # [128, 2]

            # consts ride the ACT ring so the x stream owns the SP ring;
            # ca is split so the hc0 weights (+biases) land first
            nc.scalar.dma_start(ca_sb[:, :516], ca_d[:, :516])
            nc.scalar.dma_start(ca_sb[:, 516:], ca_d[:, 516:])
            nc.scalar.dma_start(cb_sb[:, :], cb_d[:, :])

            # Group 0's x arrives as 4 per-chunk DMAs so its first L1 matmul
            # can start as soon as chunk 0 lands; later groups use one DMA.
            x0c = []
            for kc in range(4):
                t = const.tile([128, GROUP], bf16, name=f"x0c{kc}")
                nc.sync.dma_start(t[:, :], xg_d[0, :, kc * GROUP : (kc + 1) * GROUP])
                x0c.append(t)
            x_tiles = [None]
            for i, (_, _, n) in list(enumerate(groups))[1:]:
                x_sb = xp.tile([128, 4, GROUP], bf16, tag="x", name=f"x_sb{i}")
                if n == GROUP:
                    nc.sync.dma_start(x_sb.rearrange("p c t -> p (c t)"), xg_d[i])
                else:
                    nc.sync.dma_start(
                        x_sb[:, :, :n],
                        xg_d[i, :, : 4 * n].rearrange("p (c t) -> p c t", c=4),
                    )
                x_tiles.append(x_sb)

            def xin(i, kc, n):
                if i == 0:
                    return x0c[kc][:, :n]
                return x_tiles[i][:, kc, :n]

            # PE warm-up while the first x DMA is in flight: keeps the HAM
            # activity window busy from preamble-end so the clock gate opens
            # (2.4 GHz) early in the real matmul stream.
            warm_w = const.tile([128, 256], bf16)
            nc.vector.memset(warm_w[:, :], 0.0)
            warm_p = ps1.tile([128, GROUP], f32, tag="p1", name="warm_p")
            for _ in range(8):
                nc.tensor.matmul(
                    warm_p[:, :256], warm_w[:, :128], warm_w[:, :], start=True, stop=True
                )

            # Software-pipelined emission: the PE queue is in-order, so L2(g)
            # waiting on ScalarE's h(g) and L3(g) waiting on VectorE's h1(g)
            # would stall the whole stream. Emitting L1(g) | L2(g-1) | L3(g-2)
            # gives every activation a full group of matmuls to hide behind.
            h_tiles = [None] * n_groups
            h1_tiles = [None] * n_groups
            p3_tiles = [None] * n_sb

            def emit_l1(i):
                _, _, n = groups[i]
                h_sb = hp.tile([128, 2, GROUP], bf16, tag="h", name=f"h_sb{i}")
                h_tiles[i] = h_sb
                for hc in range(2):
                    p1 = ps1.tile([128, GROUP], f32, tag="p1", name=f"p1_{i}_{hc}")
                    for kc in range(4):
                        nc.tensor.matmul(
                            p1[:, :n],
                            ca_sb[:, _ca_col(hc, kc) :][:, :128],
                            xin(i, kc, n),
                            start=(kc == 0),
                            stop=(kc == 3),
                        )
                    # h = relu(psum + bs): hc0 on VectorE, hc1 on ScalarE
                    if hc == 0:
                        nc.vector.tensor_scalar(
                            h_sb[:, hc, :n],
                            p1[:, :n],
                            bs_f32[:, hc : hc + 1],
                            0.0,
                            ALU.add,
                            ALU.max,
                        )
                    else:
                        nc.scalar.activation(
                            h_sb[:, hc, :n],
                            p1[:, :n],
                            AF.Relu,
                            bias=bs_f32[:, hc : hc + 1],
                        )

            def emit_l2(i):
                e, _, n = groups[i]
                h_sb = h_tiles[i]
                p2 = ps2.tile([128, GROUP], f32, tag="p2", name=f"p2_{i}")
                for kc in range(2):
                    nc.tensor.matmul(
                        p2[:, :n],
                        cb_sb[:, CB_W1 + e * 256 + kc * 128 :][:, :128],
                        h_sb[:, kc, :n],
                        start=(kc == 0),
                        stop=(kc == 1),
                    )
                # h1 rows 0..99 = relu(psum + b1); rows 100..127 = relu(0+1)=1
                h1_sb = h1p.tile([128, GROUP], bf16, tag="h1", name=f"h1_sb{i}")
                h1_tiles[i] = h1_sb
                nc.vector.tensor_scalar(
                    h1_sb[:, :n],
                    p2[:, :n],
                    b1_f32[:, e : e + 1],
                    0.0,
                    ALU.add,
                    ALU.max,
                )

            def emit_l3(i):
                _, _, n = groups[i]
                sb, lane = divmod(i, SB)
                # one [128, 8] stationary per group, only column `lane`
                # nonzero -> psum row `lane` gets this group's outputs
                if lane == 0:
                    p3_tiles[sb] = ps3.tile([SB, GROUP], f32, tag="p3", name=f"p3_{sb}")
                p3t = p3_tiles[sb]
                last_in_sb = (i == n_groups - 1) or (lane == SB - 1)
                nc.tensor.matmul(
                    p3t[:, :n],
                    cb_sb[:, CB_W2 + 8 * i : CB_W2 + 8 * i + 8],
                    h1_tiles[i][:, :n],
                    start=(lane == 0),
                    stop=last_in_sb,
                )
                if last_in_sb:
                    w = max(
                        groups[j][2]
                        for j in range(sb * SB, min((sb + 1) * SB, n_groups))
                    )
                    if sb == n_sb - 1:
                        # final block: DVE copy + SP-ring store, so it does
                        # not serialize behind the previous block's ACT-ring
                        # copy and DMA issue
                        nc.vector.tensor_scalar_add(o2[:, sb, :w], p3t[:, :w], 0.0)
                        nc.sync.dma_start(out_d[:, sb, :w], o2[:, sb, :w])
                    else:
                        nc.scalar.copy(o2[:, sb, :w], p3t[:, :w])
                        nc.scalar.dma_start(out_d[:, sb, :w], o2[:, sb, :w])

            for g in range(n_groups + 2):
                if g < n_groups:
                    emit_l1(g)
                if 1 <= g <= n_groups:
                    emit_l2(g - 1)
                if g >= 2:
                    emit_l3(g - 2)

    nc.compile()
    return nc


def _get_program(C: int):
    if C not in _PROGRAM_CACHE:
        _PROGRAM_CACHE[C] = _build_program(C)
    return _PROGRAM_CACHE[C]


def kernel(x, idx, Ws, bs, W1, b1, W2, b2, _trace=False, _result_box=None):
    from concourse.bass_utils import run_bass_kernel_spmd

    x = np.asarray(x)
    idx = np.asarray(idx).astype(np.int64)
    Ws = np.asarray(Ws, dtype=np.float32)
    bs = np.asarray(bs, dtype=np.float32)
    W1 = np.asarray(W1, dtype=np.float32)
    b1 = np.asarray(b1, dtype=np.float32)
    W2 = np.asarray(W2, dtype=np.float32)
    b2 = np.asarray(b2, dtype=np.float32)

    counts = np.bincount(idx, minlength=N_EXP)
    C = max(GROUP, int(math.ceil(counts.max() / 128) * 128))
    nc = _get_program(C)

    groups = _make_groups(C)
    n_groups = len(groups)
    n_sb = (n_groups + SB - 1) // SB
    cb_cols = CB_W2 + 8 * n_groups

    order = np.argsort(idx, kind="stable")
    bounds = np.zeros(N_EXP + 1, dtype=np.int64)
    np.cumsum(counts, out=bounds[1:])
    tok_by_expert = [order[bounds[e] : bounds[e + 1]] for e in range(N_EXP)]

    # constA (same for every core): ws blocked so [:, kc*256+hc*128:+128] is
    # the (kc, hc) stationary; bs as [128, 2] columns
    ca = np.zeros((128, CA_COLS), dtype=BF16)
    wsb = Ws.reshape(4, 128, 2, 128).astype(BF16)  # [kc, p, hc, j]
    for hc in range(2):
        for kc in range(4):
            ca[:, _ca_col(hc, kc) : _ca_col(hc, kc) + 128] = wsb[:, :, hc][kc]
    ca[:, CA_BS : CA_BS + 4] = np.ascontiguousarray(
        bs.reshape(2, 128).T.astype(np.float32)
    ).view(BF16)

    x_bf = x.astype(BF16)
    in_maps = []
    core_tokens = []
    for c in range(N_CORES):
        ea, eb = 2 * c, 2 * c + 1
        toks = np.zeros(2 * C, dtype=np.int64)
        toks[: counts[ea]] = tok_by_expert[ea]
        toks[C : C + counts[eb]] = tok_by_expert[eb]
        core_tokens.append(toks)

        # x blocks in execution order; remainder groups packed compactly
        xg = np.zeros((n_groups, 128, 4 * GROUP), dtype=BF16)
        for i, (slot, off, n) in enumerate(groups):
            blk = x_bf[toks[slot * C + off : slot * C + off + n]]  # [n, 512]
            # [n, 4, 128] -> [128, 4, n] -> [128, 4*n]
            xg[i, :, : 4 * n] = (
                blk.reshape(n, 4, 128).transpose(2, 1, 0).reshape(128, 4 * n)
            )

        cb = np.zeros((128, cb_cols), dtype=BF16)
        w1p = np.zeros((2, HID, 128), dtype=np.float32)
        w1p[0, :, :EXP_HID] = W1[ea]
        w1p[1, :, :EXP_HID] = W1[eb]
        cb[:, CB_W1 : CB_W1 + 512] = (
            w1p.reshape(2, 2, 128, 128).transpose(2, 0, 1, 3).reshape(128, 512)
        ).astype(BF16)
        b1p = np.ones((128, 2), dtype=np.float32)
        b1p[:EXP_HID] = b1[[ea, eb]].T
        cb[:, CB_B1 : CB_B1 + 4] = np.ascontiguousarray(b1p).view(BF16)
        w2v = np.zeros((128, 2), dtype=np.float32)
        w2v[:EXP_HID] = W2[[ea, eb], :, 0].T
        w2v[EXP_HID] = b2[[ea, eb], 0]
        for i, (slot, off, n) in enumerate(groups):
            lane = i % SB
            cb[:, CB_W2 + 8 * i + lane] = w2v[:, slot].astype(BF16)

        in_maps.append({"xg": xg, "ca": ca, "cb": cb})

    res = run_bass_kernel_spmd(
        nc,
        in_maps,
        core_ids=list(range(N_CORES)),
        trace=_trace,
        **({"trace_cores": [0]} if _trace else {}),
    )
    if _result_box is not None:
        _result_box.append(res)

    out = np.zeros((B, OUT_DIM), dtype=np.float32)
    for c in range(N_CORES):
        oc = res.results[c]["out"]  # [SB, n_sb, GROUP] f32
        toks = core_tokens[c]
        for i, (slot, off, n) in enumerate(groups):
            sb, lane = divmod(i, SB)
            # skip padding rows (they alias token 0)
            v = max(0, min(n, int(counts[2 * c + slot]) - off))
            if v:
                out[toks[slot * C + off : slot * C + off + v], 0] = oc[lane, sb, :v]
    return out


# revision 13
# speedup vs baseline: 1.0358x; 1.0358x over previous
"""MoE routing kernel for 8 Trainium2 NeuronCores.

Problem: B=65536 tokens, shared Linear(512->256)+ReLU, then per-token expert
MLP Linear(256->100)+ReLU -> Linear(100->1), expert chosen by idx in [0,16).

Strategy (expert-parallel, host-side routing):
  - Host sorts tokens by expert. Experts 2c and 2c+1 go to core c, each in a
    fixed-capacity slot of C tokens (C = max expert count rounded up to 128),
    padded with token 0 (padding outputs are computed then discarded).
  - Host pre-transposes x to [512, TOK] bf16 per core so the contraction dim
    (IN_DIM) lands on SBUF partitions: all three GEMMs then chain on-chip with
    no transposes (layer-1 out [hid, tok] feeds layer-2, which feeds layer-3).
  - Device: per group of <=512 tokens: one DMA of x columns, 8 accumulating
    matmuls (512-dim contraction, 2 hid chunks) + bias/ReLU, 2 matmuls for
    expert FC1 + bias/ReLU, 1 matmul for FC2 (bias folded in via a ones-row).
  - All weights/biases ride in TWO packed contiguous bf16 DMAs on the ACT
    HWDGE ring; the SP ring carries only the x stream, so the first x group
    lands as early as possible.
  - Layer-3 outputs for 8 consecutive groups accumulate into one [8, 512]
    PSUM tile (per-group stationary is [128, 8] with a single nonzero column),
    so the store path is 3 copies + 3 multi-partition DMAs instead of a slow
    single-partition tail.
"""

import math
import os
import sys

import numpy as np

for _p in ("/opt/trn_rl_repo", "/opt/pypackages"):
    if _p not in sys.path and os.path.isdir(_p):
        sys.path.append(_p)

import ml_dtypes

BF16 = ml_dtypes.bfloat16

B, IN_DIM, HID, EXP_HID, OUT_DIM, N_EXP = 65536, 512, 256, 100, 1, 16
N_CORES = 8
GROUP = 512  # tokens per matmul group (= PSUM bank free-dim in fp32)
SB = 8  # groups per output super-block

# packed const layouts (bf16 columns per partition); biases are f32 stored
# as raw bytes (2 bf16 cols per value) and bitcast on device.
# constA is hc-major and split into two DMAs so the first L1 matmuls only
# gate on the first half: [ws hc0 512 | bs 4 | ws hc1 512]
CA_BS = 512
CA_COLS = 1028
CB_W1, CB_B1, CB_W2 = 0, 512, 516  # constB: w1 [2*2*128], b1 [2 f32], w2sel


def _ca_col(hc, kc):
    return hc * 516 + kc * 128

_PROGRAM_CACHE = {}


def _make_groups(C):
    """Execution-ordered blocks: (slot, off, n).

    Remainder (short) blocks run last: they form the final output super-block,
    so the last L3 -> copy -> store chain is short.
    """
    full, rem = [], []
    for slot in range(2):
        off = 0
        while off < C:
            n = min(GROUP, C - off)
            (full if n == GROUP else rem).append((slot, off, n))
            off += n
    return full + rem


def _build_program(C: int):
    """Build (and cache) the Bass program for per-expert-slot capacity C."""
    import concourse.bass as bass
    import concourse.mybir as mybir
    import concourse.tile as tile
    from concourse import bacc

    TOK = 2 * C
    f32 = mybir.dt.float32
    bf16 = mybir.dt.bfloat16
    AF = mybir.ActivationFunctionType
    ALU = mybir.AluOpType

    groups = _make_groups(C)
    n_groups = len(groups)
    n_sb = (n_groups + SB - 1) // SB
    cb_cols = CB_W2 + 8 * n_groups

    nc = bacc.Bacc("TRN2", target_bir_lowering=False, debug=False)

    # xg[i] holds the i-th executed group's x block: for full groups
    # xg[i, p, kc*512 + t] = x[tok, kc*128 + p]; remainder groups are packed
    # compactly as xg[i, p, kc*n + t] (contiguous HBM reads).
    xg_d = nc.dram_tensor(
        "xg", [n_groups, 128, 4 * GROUP], bf16, kind="ExternalInput"
    ).ap()
    ca_d = nc.dram_tensor("ca", [128, CA_COLS], bf16, kind="ExternalInput").ap()
    cb_d = nc.dram_tensor("cb", [128, cb_cols], bf16, kind="ExternalInput").ap()
    out_d = nc.dram_tensor("out", [SB, n_sb, GROUP], f32, kind="ExternalOutput").ap()

    with tile.TileContext(nc) as tc:
        with (
            tc.tile_pool(name="const", bufs=1) as const,
            tc.tile_pool(name="xp", bufs=8) as xp,
            tc.tile_pool(name="hp", bufs=5) as hp,
            tc.tile_pool(name="h1p", bufs=5) as h1p,
            tc.tile_pool(name="ps1", bufs=4, space="PSUM") as ps1,
            tc.tile_pool(name="ps2", bufs=2, space="PSUM") as ps2,
            tc.tile_pool(name="ps3", bufs=2, space="PSUM") as ps3,
        ):
            ca_sb = const.tile([128, CA_COLS], bf16)
            cb_sb = const.tile([128, cb_cols], bf16)
            o2 = const.tile([SB, n_sb, GROUP], f32)
            bs_f32 = ca_sb[:, CA_BS : CA_BS + 4].bitcast(f32)  # [128, 2]
            b1_f32 = cb_sb[:, CB_B1 : CB_B1 + 4].bitcast(f32)  # [128, 2]

            # ca rides the ACT ring (hc0 weights + biases land first); cb is
            # issued on the SP ring *after* x1 so it does not steal bandwidth
            # from the startup-critical x transfers (ring order is FIFO).
            nc.scalar.dma_start(ca_sb[:, :516], ca_d[:, :516])
            nc.scalar.dma_start(ca_sb[:, 516:], ca_d[:, 516:])

            # Group 0's x arrives as 4 per-chunk DMAs so its first L1 matmul
            # can start as soon as chunk 0 lands; later groups use one DMA.
            x0c = []
            for kc in range(4):
                t = const.tile([128, GROUP], bf16, name=f"x0c{kc}")
                nc.sync.dma_start(t[:, :], xg_d[0, :, kc * GROUP : (kc + 1) * GROUP])
                x0c.append(t)
            x_tiles = [None]
            for i, (_, _, n) in list(enumerate(groups))[1:]:
                x_sb = xp.tile([128, 4, GROUP], bf16, tag="x", name=f"x_sb{i}")
                if n == GROUP:
                    nc.sync.dma_start(x_sb.rearrange("p c t -> p (c t)"), xg_d[i])
                else:
                    nc.sync.dma_start(
                        x_sb[:, :, :n],
                        xg_d[i, :, : 4 * n].rearrange("p (c t) -> p c t", c=4),
                    )
                x_tiles.append(x_sb)
                if i == 1:
                    nc.sync.dma_start(cb_sb[:, :], cb_d[:, :])

            def xin(i, kc, n):
                if i == 0:
                    return x0c[kc][:, :n]
                return x_tiles[i][:, kc, :n]

            # PE warm-up while the first x DMA is in flight: keeps the HAM
            # activity window busy from preamble-end so the clock gate opens
            # (2.4 GHz) early in the real matmul stream.
            warm_w = const.tile([128, 256], bf16)
            nc.vector.memset(warm_w[:, :], 0.0)
            warm_p = ps1.tile([128, GROUP], f32, tag="p1", name="warm_p")
            for _ in range(8):
                nc.tensor.matmul(
                    warm_p[:, :256], warm_w[:, :128], warm_w[:, :], start=True, stop=True
                )

            # Software-pipelined emission: the PE queue is in-order, so L2(g)
            # waiting on ScalarE's h(g) and L3(g) waiting on VectorE's h1(g)
            # would stall the whole stream. Emitting L1(g) | L2(g-2) | L3(g-4)
            # gives every activation two full groups of matmuls to hide
            # behind, absorbing cross-engine semaphore latency too.
            h_tiles = [None] * n_groups
            h1_tiles = [None] * n_groups
            p3_tiles = [None] * n_sb

            def emit_l1(i):
                _, _, n = groups[i]
                h_sb = hp.tile([128, 2, GROUP], bf16, tag="h", name=f"h_sb{i}")
                h_tiles[i] = h_sb
                for hc in range(2):
                    p1 = ps1.tile([128, GROUP], f32, tag="p1", name=f"p1_{i}_{hc}")
                    for kc in range(4):
                        nc.tensor.matmul(
                            p1[:, :n],
                            ca_sb[:, _ca_col(hc, kc) :][:, :128],
                            xin(i, kc, n),
                            start=(kc == 0),
                            stop=(kc == 3),
                        )
                    # h = relu(psum + bs): hc0 on VectorE, hc1 on ScalarE
                    if hc == 0:
                        nc.vector.tensor_scalar(
                            h_sb[:, hc, :n],
                            p1[:, :n],
                            bs_f32[:, hc : hc + 1],
                            0.0,
                            ALU.add,
                            ALU.max,
                        )
                    else:
                        nc.scalar.activation(
                            h_sb[:, hc, :n],
                            p1[:, :n],
                            AF.Relu,
                            bias=bs_f32[:, hc : hc + 1],
                        )

            def emit_l2(i):
                e, _, n = groups[i]
                h_sb = h_tiles[i]
                p2 = ps2.tile([128, GROUP], f32, tag="p2", name=f"p2_{i}")
                for kc in range(2):
                    nc.tensor.matmul(
                        p2[:, :n],
                        cb_sb[:, CB_W1 + e * 256 + kc * 128 :][:, :128],
                        h_sb[:, kc, :n],
                        start=(kc == 0),
                        stop=(kc == 1),
                    )
                # h1 rows 0..99 = relu(psum + b1); rows 100..127 = relu(0+1)=1
                h1_sb = h1p.tile([128, GROUP], bf16, tag="h1", name=f"h1_sb{i}")
                h1_tiles[i] = h1_sb
                nc.vector.tensor_scalar(
                    h1_sb[:, :n],
                    p2[:, :n],
                    b1_f32[:, e : e + 1],
                    0.0,
                    ALU.add,
                    ALU.max,
                )

            def emit_l3(i):
                _, _, n = groups[i]
                sb, lane = divmod(i, SB)
                # one [128, 8] stationary per group, only column `lane`
                # nonzero -> psum row `lane` gets this group's outputs
                if lane == 0:
                    p3_tiles[sb] = ps3.tile([SB, GROUP], f32, tag="p3", name=f"p3_{sb}")
                p3t = p3_tiles[sb]
                last_in_sb = (i == n_groups - 1) or (lane == SB - 1)
                nc.tensor.matmul(
                    p3t[:, :n],
                    cb_sb[:, CB_W2 + 8 * i : CB_W2 + 8 * i + 8],
                    h1_tiles[i][:, :n],
                    start=(lane == 0),
                    stop=last_in_sb,
                )
                if last_in_sb:
                    w = max(
                        groups[j][2]
                        for j in range(sb * SB, min((sb + 1) * SB, n_groups))
                    )
                    if sb == n_sb - 1:
                        # final block: DVE copy + SP-ring store, so it does
                        # not serialize behind the previous block's ACT-ring
                        # copy and DMA issue
                        nc.vector.tensor_scalar_add(o2[:, sb, :w], p3t[:, :w], 0.0)
                        nc.sync.dma_start(out_d[:, sb, :w], o2[:, sb, :w])
                    else:
                        nc.scalar.copy(o2[:, sb, :w], p3t[:, :w])
                        nc.scalar.dma_start(out_d[:, sb, :w], o2[:, sb, :w])

            for g in range(n_groups + 4):
                if g < n_groups:
                    emit_l1(g)
                if 2 <= g < n_groups + 2:
                    emit_l2(g - 2)
                if g >= 4:
                    emit_l3(g - 4)

    nc.compile()
    return nc


def _get_program(C: int):
    if C not in _PROGRAM_CACHE:
        _PROGRAM_CACHE[C] = _build_program(C)
    return _PROGRAM_CACHE[C]


def kernel(x, idx, Ws, bs, W1, b1, W2, b2, _trace=False, _result_box=None):
    from concourse.bass_utils import run_bass_kernel_spmd

    x = np.asarray(x)
    idx = np.asarray(idx).astype(np.int64)
    Ws = np.asarray(Ws, dtype=np.float32)
    bs = np.asarray(bs, dtype=np.float32)
    W1 = np.asarray(W1, dtype=np.float32)
    b1 = np.asarray(b1, dtype=np.float32)
    W2 = np.asarray(W2, dtype=np.float32)
    b2 = np.asarray(b2, dtype=np.float32)

    counts = np.bincount(idx, minlength=N_EXP)
    C = max(GROUP, int(math.ceil(counts.max() / 128) * 128))
    nc = _get_program(C)

    groups = _make_groups(C)
    n_groups = len(groups)
    n_sb = (n_groups + SB - 1) // SB
    cb_cols = CB_W2 + 8 * n_groups

    order = np.argsort(idx, kind="stable")
    bounds = np.zeros(N_EXP + 1, dtype=np.int64)
    np.cumsum(counts, out=bounds[1:])
    tok_by_expert = [order[bounds[e] : bounds[e + 1]] for e in range(N_EXP)]

    # constA (same for every core): ws blocked so [:, kc*256+hc*128:+128] is
    # the (kc, hc) stationary; bs as [128, 2] columns
    ca = np.zeros((128, CA_COLS), dtype=BF16)
    wsb = Ws.reshape(4, 128, 2, 128).astype(BF16)  # [kc, p, hc, j]
    for hc in range(2):
        for kc in range(4):
            ca[:, _ca_col(hc, kc) : _ca_col(hc, kc) + 128] = wsb[:, :, hc][kc]
    ca[:, CA_BS : CA_BS + 4] = np.ascontiguousarray(
        bs.reshape(2, 128).T.astype(np.float32)
    ).view(BF16)

    x_bf = x.astype(BF16)
    in_maps = []
    core_tokens = []
    for c in range(N_CORES):
        ea, eb = 2 * c, 2 * c + 1
        toks = np.zeros(2 * C, dtype=np.int64)
        toks[: counts[ea]] = tok_by_expert[ea]
        toks[C : C + counts[eb]] = tok_by_expert[eb]
        core_tokens.append(toks)

        # x blocks in execution order; remainder groups packed compactly
        xg = np.zeros((n_groups, 128, 4 * GROUP), dtype=BF16)
        for i, (slot, off, n) in enumerate(groups):
            blk = x_bf[toks[slot * C + off : slot * C + off + n]]  # [n, 512]
            # [n, 4, 128] -> [128, 4, n] -> [128, 4*n]
            xg[i, :, : 4 * n] = (
                blk.reshape(n, 4, 128).transpose(2, 1, 0).reshape(128, 4 * n)
            )

        cb = np.zeros((128, cb_cols), dtype=BF16)
        w1p = np.zeros((2, HID, 128), dtype=np.float32)
        w1p[0, :, :EXP_HID] = W1[ea]
        w1p[1, :, :EXP_HID] = W1[eb]
        cb[:, CB_W1 : CB_W1 + 512] = (
            w1p.reshape(2, 2, 128, 128).transpose(2, 0, 1, 3).reshape(128, 512)
        ).astype(BF16)
        b1p = np.ones((128, 2), dtype=np.float32)
        b1p[:EXP_HID] = b1[[ea, eb]].T
        cb[:, CB_B1 : CB_B1 + 4] = np.ascontiguousarray(b1p).view(BF16)
        w2v = np.zeros((128, 2), dtype=np.float32)
        w2v[:EXP_HID] = W2[[ea, eb], :, 0].T
        w2v[EXP_HID] = b2[[ea, eb], 0]
        for i, (slot, off, n) in enumerate(groups):
            lane = i % SB
            cb[:, CB_W2 + 8 * i + lane] = w2v[:, slot].astype(BF16)

        in_maps.append({"xg": xg, "ca": ca, "cb": cb})

    res = run_bass_kernel_spmd(
        nc,
        in_maps,
        core_ids=list(range(N_CORES)),
        trace=_trace,
        **({"trace_cores": [0]} if _trace else {}),
    )
    if _result_box is not None:
        _result_box.append(res)

    out = np.zeros((B, OUT_DIM), dtype=np.float32)
    for c in range(N_CORES):
        oc = res.results[c]["out"]  # [SB, n_sb, GROUP] f32
        toks = core_tokens[c]
        for i, (slot, off, n) in enumerate(groups):
            sb, lane = divmod(i, SB)
            # skip padding rows (they alias token 0)
            v = max(0, min(n, int(counts[2 * c + slot]) - off))
            if v:
                out[toks[slot * C + off : slot * C + off + v], 0] = oc[lane, sb, :v]
    return out


# revision 15
# speedup vs baseline: 1.0455x; 1.0094x over previous
"""MoE routing kernel for 8 Trainium2 NeuronCores.

Problem: B=65536 tokens, shared Linear(512->256)+ReLU, then per-token expert
MLP Linear(256->100)+ReLU -> Linear(100->1), expert chosen by idx in [0,16).

Strategy (expert-parallel, host-side routing):
  - Host sorts tokens by expert. Experts 2c and 2c+1 go to core c, each in a
    fixed-capacity slot of C tokens (C = max expert count rounded up to 128),
    padded with token 0 (padding outputs are computed then discarded).
  - Host pre-transposes x to [512, TOK] bf16 per core so the contraction dim
    (IN_DIM) lands on SBUF partitions: all three GEMMs then chain on-chip with
    no transposes (layer-1 out [hid, tok] feeds layer-2, which feeds layer-3).
  - Device: per group of <=512 tokens: one DMA of x columns, 8 accumulating
    matmuls (512-dim contraction, 2 hid chunks) + bias/ReLU, 2 matmuls for
    expert FC1 + bias/ReLU, 1 matmul for FC2 (bias folded in via a ones-row).
  - All weights/biases ride in TWO packed contiguous bf16 DMAs on the ACT
    HWDGE ring; the SP ring carries only the x stream, so the first x group
    lands as early as possible.
  - Layer-3 outputs for 8 consecutive groups accumulate into one [8, 512]
    PSUM tile (per-group stationary is [128, 8] with a single nonzero column),
    so the store path is 3 copies + 3 multi-partition DMAs instead of a slow
    single-partition tail.
"""

import math
import os
import sys

import numpy as np

for _p in ("/opt/trn_rl_repo", "/opt/pypackages"):
    if _p not in sys.path and os.path.isdir(_p):
        sys.path.append(_p)

import ml_dtypes

BF16 = ml_dtypes.bfloat16

B, IN_DIM, HID, EXP_HID, OUT_DIM, N_EXP = 65536, 512, 256, 100, 1, 16
N_CORES = 8
GROUP = 512  # tokens per matmul group (= PSUM bank free-dim in fp32)
SB = 8  # groups per output super-block

# packed const layouts (bf16 columns per partition); biases are f32 stored
# as raw bytes (2 bf16 cols per value) and bitcast on device.
# constA is hc-major and split into two DMAs so the first L1 matmuls only
# gate on the first half: [ws hc0 512 | bs 4 | ws hc1 512]
CA_BS = 512
CA_COLS = 1028
CB_W1, CB_B1, CB_W2 = 0, 512, 516  # constB: w1 [2*2*128], b1 [2 f32], w2sel
# w2sel is one 128-wide block per group (only column `lane` nonzero) so its
# LDWEIGHTS is FWL-eligible and hides behind the preceding matmul


def _ca_col(hc, kc):
    return hc * 516 + kc * 128

_PROGRAM_CACHE = {}


def _make_groups(C):
    """Execution-ordered blocks: (slot, off, n).

    Remainder (short) blocks run last: they form the final output super-block,
    so the last L3 -> copy -> store chain is short.
    """
    full, rem = [], []
    for slot in range(2):
        off = 0
        while off < C:
            n = min(GROUP, C - off)
            (full if n == GROUP else rem).append((slot, off, n))
            off += n
    return full + rem


def _build_program(C: int):
    """Build (and cache) the Bass program for per-expert-slot capacity C."""
    import concourse.bass as bass
    import concourse.mybir as mybir
    import concourse.tile as tile
    from concourse import bacc

    TOK = 2 * C
    f32 = mybir.dt.float32
    bf16 = mybir.dt.bfloat16
    AF = mybir.ActivationFunctionType
    ALU = mybir.AluOpType

    groups = _make_groups(C)
    n_groups = len(groups)
    n_sb = (n_groups + SB - 1) // SB
    cb_cols = CB_W2 + 128 * n_groups

    nc = bacc.Bacc("TRN2", target_bir_lowering=False, debug=False)

    # xg[i] holds the i-th executed group's x block: for full groups
    # xg[i, p, kc*512 + t] = x[tok, kc*128 + p]; remainder groups are packed
    # compactly as xg[i, p, kc*n + t] (contiguous HBM reads).
    xg_d = nc.dram_tensor(
        "xg", [n_groups, 128, 4 * GROUP], bf16, kind="ExternalInput"
    ).ap()
    ca_d = nc.dram_tensor("ca", [128, CA_COLS], bf16, kind="ExternalInput").ap()
    cb_d = nc.dram_tensor("cb", [128, cb_cols], bf16, kind="ExternalInput").ap()
    out_d = nc.dram_tensor("out", [SB, n_sb, GROUP], f32, kind="ExternalOutput").ap()

    with tile.TileContext(nc) as tc:
        with (
            tc.tile_pool(name="const", bufs=1) as const,
            tc.tile_pool(name="xp", bufs=8) as xp,
            tc.tile_pool(name="hp", bufs=5) as hp,
            tc.tile_pool(name="h1p", bufs=5) as h1p,
            tc.tile_pool(name="ps1", bufs=5, space="PSUM") as ps1,
            tc.tile_pool(name="ps2", bufs=2, space="PSUM") as ps2,
            tc.tile_pool(name="ps3", bufs=1, space="PSUM") as ps3,
        ):
            ca_sb = const.tile([128, CA_COLS], bf16)
            cb_sb = const.tile([128, cb_cols], bf16)
            o2 = const.tile([SB, n_sb, GROUP], f32)
            bs_f32 = ca_sb[:, CA_BS : CA_BS + 4].bitcast(f32)  # [128, 2]
            b1_f32 = cb_sb[:, CB_B1 : CB_B1 + 4].bitcast(f32)  # [128, 2]

            # ca rides the ACT ring (hc0 weights + biases land first); cb is
            # issued on the SP ring *after* x1 so it does not steal bandwidth
            # from the startup-critical x transfers (ring order is FIFO).
            nc.scalar.dma_start(ca_sb[:, :516], ca_d[:, :516])
            nc.scalar.dma_start(ca_sb[:, 516:], ca_d[:, 516:])

            # Group 0's x arrives as 4 per-chunk DMAs so its first L1 matmul
            # can start as soon as chunk 0 lands; later groups use one DMA.
            x0c = []
            for kc in range(4):
                t = const.tile([128, GROUP], bf16, name=f"x0c{kc}")
                nc.sync.dma_start(t[:, :], xg_d[0, :, kc * GROUP : (kc + 1) * GROUP])
                x0c.append(t)
            x_tiles = [None]
            for i, (_, _, n) in list(enumerate(groups))[1:]:
                x_sb = xp.tile([128, 4, GROUP], bf16, tag="x", name=f"x_sb{i}")
                if n == GROUP:
                    nc.sync.dma_start(x_sb.rearrange("p c t -> p (c t)"), xg_d[i])
                else:
                    nc.sync.dma_start(
                        x_sb[:, :, :n],
                        xg_d[i, :, : 4 * n].rearrange("p (c t) -> p c t", c=4),
                    )
                x_tiles.append(x_sb)
                if i == 1:
                    nc.sync.dma_start(cb_sb[:, :CB_W2], cb_d[:, :CB_W2])
                if i == 3:
                    nc.sync.dma_start(cb_sb[:, CB_W2:], cb_d[:, CB_W2:])

            def xin(i, kc, n):
                if i == 0:
                    return x0c[kc][:, :n]
                return x_tiles[i][:, kc, :n]

            # PE warm-up while the first x DMA is in flight: keeps the HAM
            # activity window busy from preamble-end so the clock gate opens
            # (2.4 GHz) early in the real matmul stream.
            warm_w = const.tile([128, 256], bf16)
            nc.vector.memset(warm_w[:, :], 0.0)
            warm_p = ps1.tile([128, GROUP], f32, tag="p1", name="warm_p")
            for _ in range(12):
                nc.tensor.matmul(
                    warm_p[:, :256], warm_w[:, :128], warm_w[:, :], start=True, stop=True
                )

            # Software-pipelined emission: the PE queue is in-order, so L2(g)
            # waiting on ScalarE's h(g) and L3(g) waiting on VectorE's h1(g)
            # would stall the whole stream. Emitting L1(g) | L2(g-2) | L3(g-4)
            # gives every activation two full groups of matmuls to hide
            # behind, absorbing cross-engine semaphore latency too.
            h_tiles = [None] * n_groups
            h1_tiles = [None] * n_groups
            p3_tiles = [None] * n_sb

            def emit_l1(i):
                _, _, n = groups[i]
                h_sb = hp.tile([128, 2, GROUP], bf16, tag="h", name=f"h_sb{i}")
                h_tiles[i] = h_sb
                for hc in range(2):
                    p1 = ps1.tile([128, GROUP], f32, tag="p1", name=f"p1_{i}_{hc}")
                    for kc in range(4):
                        nc.tensor.matmul(
                            p1[:, :n],
                            ca_sb[:, _ca_col(hc, kc) :][:, :128],
                            xin(i, kc, n),
                            start=(kc == 0),
                            stop=(kc == 3),
                        )
                    # h = relu(psum + bs): hc0 on VectorE, hc1 on ScalarE
                    if hc == 0:
                        nc.vector.tensor_scalar(
                            h_sb[:, hc, :n],
                            p1[:, :n],
                            bs_f32[:, hc : hc + 1],
                            0.0,
                            ALU.add,
                            ALU.max,
                        )
                    else:
                        nc.scalar.activation(
                            h_sb[:, hc, :n],
                            p1[:, :n],
                            AF.Relu,
                            bias=bs_f32[:, hc : hc + 1],
                        )

            def emit_l2(i):
                e, _, n = groups[i]
                h_sb = h_tiles[i]
                p2 = ps2.tile([128, GROUP], f32, tag="p2", name=f"p2_{i}")
                for kc in range(2):
                    nc.tensor.matmul(
                        p2[:, :n],
                        cb_sb[:, CB_W1 + e * 256 + kc * 128 :][:, :128],
                        h_sb[:, kc, :n],
                        start=(kc == 0),
                        stop=(kc == 1),
                    )
                # h1 rows 0..99 = relu(psum + b1); rows 100..127 = relu(0+1)=1
                h1_sb = h1p.tile([128, GROUP], bf16, tag="h1", name=f"h1_sb{i}")
                h1_tiles[i] = h1_sb
                nc.vector.tensor_scalar(
                    h1_sb[:, :n],
                    p2[:, :n],
                    b1_f32[:, e : e + 1],
                    0.0,
                    ALU.add,
                    ALU.max,
                )

            def emit_l3(i):
                _, _, n = groups[i]
                sb, lane = divmod(i, SB)
                # one [128, 8] stationary per group, only column `lane`
                # nonzero -> psum row `lane` gets this group's outputs
                if lane == 0:
                    p3_tiles[sb] = ps3.tile([128, GROUP], f32, tag="p3", name=f"p3_{sb}")
                p3t = p3_tiles[sb]
                last_in_sb = (i == n_groups - 1) or (lane == SB - 1)
                nc.tensor.matmul(
                    p3t[:, :n],
                    cb_sb[:, CB_W2 + 128 * i : CB_W2 + 128 * i + 128],
                    h1_tiles[i][:, :n],
                    start=(lane == 0),
                    stop=last_in_sb,
                )
                if last_in_sb:
                    w = max(
                        groups[j][2]
                        for j in range(sb * SB, min((sb + 1) * SB, n_groups))
                    )
                    if sb == n_sb - 1:
                        # final block: DVE copy + SP-ring store, so it does
                        # not serialize behind the previous block's ACT-ring
                        # copy and DMA issue
                        nc.vector.tensor_scalar_add(o2[:, sb, :w], p3t[:8, :w], 0.0)
                        nc.sync.dma_start(out_d[:, sb, :w], o2[:, sb, :w])
                    else:
                        nc.scalar.copy(o2[:, sb, :w], p3t[:8, :w])
                        nc.scalar.dma_start(out_d[:, sb, :w], o2[:, sb, :w])

            for g in range(n_groups + 4):
                if g < n_groups:
                    emit_l1(g)
                if 2 <= g < n_groups + 2:
                    emit_l2(g - 2)
                if g >= 4:
                    emit_l3(g - 4)

    nc.compile()
    return nc


def _get_program(C: int):
    if C not in _PROGRAM_CACHE:
        _PROGRAM_CACHE[C] = _build_program(C)
    return _PROGRAM_CACHE[C]


def kernel(x, idx, Ws, bs, W1, b1, W2, b2, _trace=False, _result_box=None):
    from concourse.bass_utils import run_bass_kernel_spmd

    x = np.asarray(x)
    idx = np.asarray(idx).astype(np.int64)
    Ws = np.asarray(Ws, dtype=np.float32)
    bs = np.asarray(bs, dtype=np.float32)
    W1 = np.asarray(W1, dtype=np.float32)
    b1 = np.asarray(b1, dtype=np.float32)
    W2 = np.asarray(W2, dtype=np.float32)
    b2 = np.asarray(b2, dtype=np.float32)

    counts = np.bincount(idx, minlength=N_EXP)
    C = max(GROUP, int(math.ceil(counts.max() / 128) * 128))
    nc = _get_program(C)

    groups = _make_groups(C)
    n_groups = len(groups)
    n_sb = (n_groups + SB - 1) // SB
    cb_cols = CB_W2 + 128 * n_groups

    order = np.argsort(idx, kind="stable")
    bounds = np.zeros(N_EXP + 1, dtype=np.int64)
    np.cumsum(counts, out=bounds[1:])
    tok_by_expert = [order[bounds[e] : bounds[e + 1]] for e in range(N_EXP)]

    # constA (same for every core): ws blocked so [:, kc*256+hc*128:+128] is
    # the (kc, hc) stationary; bs as [128, 2] columns
    ca = np.zeros((128, CA_COLS), dtype=BF16)
    wsb = Ws.reshape(4, 128, 2, 128).astype(BF16)  # [kc, p, hc, j]
    for hc in range(2):
        for kc in range(4):
            ca[:, _ca_col(hc, kc) : _ca_col(hc, kc) + 128] = wsb[:, :, hc][kc]
    ca[:, CA_BS : CA_BS + 4] = np.ascontiguousarray(
        bs.reshape(2, 128).T.astype(np.float32)
    ).view(BF16)

    x_bf = x.astype(BF16)
    in_maps = []
    core_tokens = []
    for c in range(N_CORES):
        ea, eb = 2 * c, 2 * c + 1
        toks = np.zeros(2 * C, dtype=np.int64)
        toks[: counts[ea]] = tok_by_expert[ea]
        toks[C : C + counts[eb]] = tok_by_expert[eb]
        core_tokens.append(toks)

        # x blocks in execution order; remainder groups packed compactly
        xg = np.zeros((n_groups, 128, 4 * GROUP), dtype=BF16)
        for i, (slot, off, n) in enumerate(groups):
            blk = x_bf[toks[slot * C + off : slot * C + off + n]]  # [n, 512]
            # [n, 4, 128] -> [128, 4, n] -> [128, 4*n]
            xg[i, :, : 4 * n] = (
                blk.reshape(n, 4, 128).transpose(2, 1, 0).reshape(128, 4 * n)
            )

        cb = np.zeros((128, cb_cols), dtype=BF16)
        w1p = np.zeros((2, HID, 128), dtype=np.float32)
        w1p[0, :, :EXP_HID] = W1[ea]
        w1p[1, :, :EXP_HID] = W1[eb]
        cb[:, CB_W1 : CB_W1 + 512] = (
            w1p.reshape(2, 2, 128, 128).transpose(2, 0, 1, 3).reshape(128, 512)
        ).astype(BF16)
        b1p = np.ones((128, 2), dtype=np.float32)
        b1p[:EXP_HID] = b1[[ea, eb]].T
        cb[:, CB_B1 : CB_B1 + 4] = np.ascontiguousarray(b1p).view(BF16)
        w2v = np.zeros((128, 2), dtype=np.float32)
        w2v[:EXP_HID] = W2[[ea, eb], :, 0].T
        w2v[EXP_HID] = b2[[ea, eb], 0]
        for i, (slot, off, n) in enumerate(groups):
            lane = i % SB
            cb[:, CB_W2 + 128 * i + lane] = w2v[:, slot].astype(BF16)

        in_maps.append({"xg": xg, "ca": ca, "cb": cb})

    res = run_bass_kernel_spmd(
        nc,
        in_maps,
        core_ids=list(range(N_CORES)),
        trace=_trace,
        **({"trace_cores": [0]} if _trace else {}),
    )
    if _result_box is not None:
        _result_box.append(res)

    out = np.zeros((B, OUT_DIM), dtype=np.float32)
    for c in range(N_CORES):
        oc = res.results[c]["out"]  # [SB, n_sb, GROUP] f32
        toks = core_tokens[c]
        for i, (slot, off, n) in enumerate(groups):
            sb, lane = divmod(i, SB)
            # skip padding rows (they alias token 0)
            v = max(0, min(n, int(counts[2 * c + slot]) - off))
            if v:
                out[toks[slot * C + off : slot * C + off + v], 0] = oc[lane, sb, :v]
    return out


# revision 16
# speedup vs baseline: 1.0501x; 1.0044x over previous
"""MoE routing kernel for 8 Trainium2 NeuronCores.

Problem: B=65536 tokens, shared Linear(512->256)+ReLU, then per-token expert
MLP Linear(256->100)+ReLU -> Linear(100->1), expert chosen by idx in [0,16).

Strategy (expert-parallel, host-side routing):
  - Host sorts tokens by expert. Experts 2c and 2c+1 go to core c, each in a
    fixed-capacity slot of C tokens (C = max expert count rounded up to 128),
    padded with token 0 (padding outputs are computed then discarded).
  - Host pre-transposes x to [512, TOK] bf16 per core so the contraction dim
    (IN_DIM) lands on SBUF partitions: all three GEMMs then chain on-chip with
    no transposes (layer-1 out [hid, tok] feeds layer-2, which feeds layer-3).
  - Device: per group of <=512 tokens: one DMA of x columns, 8 accumulating
    matmuls (512-dim contraction, 2 hid chunks) + bias/ReLU, 2 matmuls for
    expert FC1 + bias/ReLU, 1 matmul for FC2 (bias folded in via a ones-row).
  - All weights/biases ride in TWO packed contiguous bf16 DMAs on the ACT
    HWDGE ring; the SP ring carries only the x stream, so the first x group
    lands as early as possible.
  - Layer-3 outputs for 8 consecutive groups accumulate into one [8, 512]
    PSUM tile (per-group stationary is [128, 8] with a single nonzero column),
    so the store path is 3 copies + 3 multi-partition DMAs instead of a slow
    single-partition tail.
"""

import math
import os
import sys

import numpy as np

for _p in ("/opt/trn_rl_repo", "/opt/pypackages"):
    if _p not in sys.path and os.path.isdir(_p):
        sys.path.append(_p)

import ml_dtypes

BF16 = ml_dtypes.bfloat16

B, IN_DIM, HID, EXP_HID, OUT_DIM, N_EXP = 65536, 512, 256, 100, 1, 16
N_CORES = 8
GROUP = 512  # tokens per matmul group (= PSUM bank free-dim in fp32)
SB = 8  # groups per output super-block

# packed const layouts (bf16 columns per partition); biases are f32 stored
# as raw bytes (2 bf16 cols per value) and bitcast on device.
# constA is hc-major and split into two DMAs so the first L1 matmuls only
# gate on the first half: [ws hc0 512 | bs 4 | ws hc1 512]
CA_BS = 512
CA_COLS = 1028
CB_W1, CB_B1, CB_W2 = 0, 512, 516  # constB: w1 [2*2*128], b1 [2 f32], w2sel
# w2sel is one 128-wide block per group (only column `lane` nonzero) so its
# LDWEIGHTS is FWL-eligible and hides behind the preceding matmul


def _ca_col(hc, kc):
    return hc * 516 + kc * 128

_PROGRAM_CACHE = {}


def _make_groups(C):
    """Execution-ordered blocks: (slot, off, n).

    Remainder (short) blocks run last: they form the final output super-block,
    so the last L3 -> copy -> store chain is short.
    """
    full, rem = [], []
    for slot in range(2):
        off = 0
        while off < C:
            n = min(GROUP, C - off)
            (full if n == GROUP else rem).append((slot, off, n))
            off += n
    return full + rem


def _build_program(C: int):
    """Build (and cache) the Bass program for per-expert-slot capacity C."""
    import concourse.bass as bass
    import concourse.mybir as mybir
    import concourse.tile as tile
    from concourse import bacc

    TOK = 2 * C
    f32 = mybir.dt.float32
    bf16 = mybir.dt.bfloat16
    AF = mybir.ActivationFunctionType
    ALU = mybir.AluOpType

    groups = _make_groups(C)
    n_groups = len(groups)
    n_sb = (n_groups + SB - 1) // SB
    cb_cols = CB_W2 + 128 * n_groups

    nc = bacc.Bacc("TRN2", target_bir_lowering=False, debug=False)

    # xg[i] holds the i-th executed group's x block: for full groups
    # xg[i, p, kc*512 + t] = x[tok, kc*128 + p]; remainder groups are packed
    # compactly as xg[i, p, kc*n + t] (contiguous HBM reads).
    xg_d = nc.dram_tensor(
        "xg", [n_groups, 128, 4 * GROUP], bf16, kind="ExternalInput"
    ).ap()
    ca_d = nc.dram_tensor("ca", [128, CA_COLS], bf16, kind="ExternalInput").ap()
    cb_d = nc.dram_tensor("cb", [128, cb_cols], bf16, kind="ExternalInput").ap()
    out_d = nc.dram_tensor("out", [SB, n_sb, GROUP], f32, kind="ExternalOutput").ap()

    with tile.TileContext(nc) as tc:
        with (
            tc.tile_pool(name="const", bufs=1) as const,
            tc.tile_pool(name="xp", bufs=8) as xp,
            tc.tile_pool(name="hp", bufs=5) as hp,
            tc.tile_pool(name="h1p", bufs=5) as h1p,
            tc.tile_pool(name="ps1", bufs=5, space="PSUM") as ps1,
            tc.tile_pool(name="ps2", bufs=1, space="PSUM") as ps2,
            tc.tile_pool(name="ps3", bufs=2, space="PSUM") as ps3,
        ):
            ca_sb = const.tile([128, CA_COLS], bf16)
            cb_sb = const.tile([128, cb_cols], bf16)
            o2 = const.tile([SB, n_sb, GROUP], f32)
            bs_f32 = ca_sb[:, CA_BS : CA_BS + 4].bitcast(f32)  # [128, 2]
            b1_f32 = cb_sb[:, CB_B1 : CB_B1 + 4].bitcast(f32)  # [128, 2]

            # ca rides the ACT ring (hc0 weights + biases land first); cb is
            # issued on the SP ring *after* x1 so it does not steal bandwidth
            # from the startup-critical x transfers (ring order is FIFO).
            nc.scalar.dma_start(ca_sb[:, :516], ca_d[:, :516])
            nc.scalar.dma_start(ca_sb[:, 516:], ca_d[:, 516:])

            x_tiles = []
            for i, (_, _, n) in enumerate(groups):
                x_sb = xp.tile([128, 4, GROUP], bf16, tag="x", name=f"x_sb{i}")
                if n == GROUP:
                    nc.sync.dma_start(x_sb.rearrange("p c t -> p (c t)"), xg_d[i])
                else:
                    nc.sync.dma_start(
                        x_sb[:, :, :n],
                        xg_d[i, :, : 4 * n].rearrange("p (c t) -> p c t", c=4),
                    )
                x_tiles.append(x_sb)
                if i == 1:
                    nc.sync.dma_start(cb_sb[:, :CB_W2], cb_d[:, :CB_W2])
                if i == 3:
                    nc.sync.dma_start(cb_sb[:, CB_W2:], cb_d[:, CB_W2:])

            def xin(i, kc, n):
                return x_tiles[i][:, kc, :n]

            # PE warm-up while the first x DMA is in flight: keeps the HAM
            # activity window busy from preamble-end so the clock gate opens
            # (2.4 GHz) early in the real matmul stream.
            warm_w = const.tile([128, 256], bf16)
            nc.vector.memset(warm_w[:, :], 0.0)
            warm_p = ps1.tile([128, GROUP], f32, tag="p1", name="warm_p")
            for _ in range(14):
                nc.tensor.matmul(
                    warm_p[:, :256], warm_w[:, :128], warm_w[:, :], start=True, stop=True
                )

            # Software-pipelined emission: the PE queue is in-order, so L2(g)
            # waiting on ScalarE's h(g) and L3(g) waiting on VectorE's h1(g)
            # would stall the whole stream. Emitting L1(g) | L2(g-2) | L3(g-4)
            # gives every activation two full groups of matmuls to hide
            # behind, absorbing cross-engine semaphore latency too.
            h_tiles = [None] * n_groups
            h1_tiles = [None] * n_groups
            p3_tiles = [None] * n_sb

            def emit_l1(i):
                _, _, n = groups[i]
                h_sb = hp.tile([128, 2, GROUP], bf16, tag="h", name=f"h_sb{i}")
                h_tiles[i] = h_sb
                for hc in range(2):
                    p1 = ps1.tile([128, GROUP], f32, tag="p1", name=f"p1_{i}_{hc}")
                    for kc in range(4):
                        nc.tensor.matmul(
                            p1[:, :n],
                            ca_sb[:, _ca_col(hc, kc) :][:, :128],
                            xin(i, kc, n),
                            start=(kc == 0),
                            stop=(kc == 3),
                        )
                    # h = relu(psum + bs): hc0 on VectorE, hc1 on ScalarE
                    if hc == 0:
                        nc.vector.tensor_scalar(
                            h_sb[:, hc, :n],
                            p1[:, :n],
                            bs_f32[:, hc : hc + 1],
                            0.0,
                            ALU.add,
                            ALU.max,
                        )
                    else:
                        nc.scalar.activation(
                            h_sb[:, hc, :n],
                            p1[:, :n],
                            AF.Relu,
                            bias=bs_f32[:, hc : hc + 1],
                        )

            def emit_l2(i):
                e, _, n = groups[i]
                h_sb = h_tiles[i]
                p2 = ps2.tile([128, GROUP], f32, tag="p2", name=f"p2_{i}")
                for kc in range(2):
                    nc.tensor.matmul(
                        p2[:, :n],
                        cb_sb[:, CB_W1 + e * 256 + kc * 128 :][:, :128],
                        h_sb[:, kc, :n],
                        start=(kc == 0),
                        stop=(kc == 1),
                    )
                # h1 rows 0..99 = relu(psum + b1); rows 100..127 = relu(0+1)=1
                h1_sb = h1p.tile([128, GROUP], bf16, tag="h1", name=f"h1_sb{i}")
                h1_tiles[i] = h1_sb
                nc.vector.tensor_scalar(
                    h1_sb[:, :n],
                    p2[:, :n],
                    b1_f32[:, e : e + 1],
                    0.0,
                    ALU.add,
                    ALU.max,
                )

            def emit_l3(i):
                _, _, n = groups[i]
                sb, lane = divmod(i, SB)
                # one [128, 8] stationary per group, only column `lane`
                # nonzero -> psum row `lane` gets this group's outputs
                if lane == 0:
                    p3_tiles[sb] = ps3.tile([128, GROUP], f32, tag="p3", name=f"p3_{sb}")
                p3t = p3_tiles[sb]
                last_in_sb = (i == n_groups - 1) or (lane == SB - 1)
                nc.tensor.matmul(
                    p3t[:, :n],
                    cb_sb[:, CB_W2 + 128 * i : CB_W2 + 128 * i + 128],
                    h1_tiles[i][:, :n],
                    start=(lane == 0),
                    stop=last_in_sb,
                )
                if last_in_sb:
                    w = max(
                        groups[j][2]
                        for j in range(sb * SB, min((sb + 1) * SB, n_groups))
                    )
                    if sb == n_sb - 1:
                        # final block: DVE copy + SP-ring store, so it does
                        # not serialize behind the previous block's ACT-ring
                        # copy and DMA issue
                        nc.vector.tensor_scalar_add(o2[:, sb, :w], p3t[:8, :w], 0.0)
                        nc.sync.dma_start(out_d[:, sb, :w], o2[:, sb, :w])
                    else:
                        nc.scalar.copy(o2[:, sb, :w], p3t[:8, :w])
                        nc.scalar.dma_start(out_d[:, sb, :w], o2[:, sb, :w])

            for g in range(n_groups + 4):
                if g < n_groups:
                    emit_l1(g)
                if 2 <= g < n_groups + 2:
                    emit_l2(g - 2)
                if g >= 4:
                    emit_l3(g - 4)

    nc.compile()
    return nc


def _get_program(C: int):
    if C not in _PROGRAM_CACHE:
        _PROGRAM_CACHE[C] = _build_program(C)
    return _PROGRAM_CACHE[C]


def kernel(x, idx, Ws, bs, W1, b1, W2, b2, _trace=False, _result_box=None):
    from concourse.bass_utils import run_bass_kernel_spmd

    x = np.asarray(x)
    idx = np.asarray(idx).astype(np.int64)
    Ws = np.asarray(Ws, dtype=np.float32)
    bs = np.asarray(bs, dtype=np.float32)
    W1 = np.asarray(W1, dtype=np.float32)
    b1 = np.asarray(b1, dtype=np.float32)
    W2 = np.asarray(W2, dtype=np.float32)
    b2 = np.asarray(b2, dtype=np.float32)

    counts = np.bincount(idx, minlength=N_EXP)
    C = max(GROUP, int(math.ceil(counts.max() / 128) * 128))
    nc = _get_program(C)

    groups = _make_groups(C)
    n_groups = len(groups)
    n_sb = (n_groups + SB - 1) // SB
    cb_cols = CB_W2 + 128 * n_groups

    order = np.argsort(idx, kind="stable")
    bounds = np.zeros(N_EXP + 1, dtype=np.int64)
    np.cumsum(counts, out=bounds[1:])
    tok_by_expert = [order[bounds[e] : bounds[e + 1]] for e in range(N_EXP)]

    # constA (same for every core): ws blocked so [:, kc*256+hc*128:+128] is
    # the (kc, hc) stationary; bs as [128, 2] columns
    ca = np.zeros((128, CA_COLS), dtype=BF16)
    wsb = Ws.reshape(4, 128, 2, 128).astype(BF16)  # [kc, p, hc, j]
    for hc in range(2):
        for kc in range(4):
            ca[:, _ca_col(hc, kc) : _ca_col(hc, kc) + 128] = wsb[:, :, hc][kc]
    ca[:, CA_BS : CA_BS + 4] = np.ascontiguousarray(
        bs.reshape(2, 128).T.astype(np.float32)
    ).view(BF16)

    x_bf = x.astype(BF16)
    in_maps = []
    core_tokens = []
    for c in range(N_CORES):
        ea, eb = 2 * c, 2 * c + 1
        toks = np.zeros(2 * C, dtype=np.int64)
        toks[: counts[ea]] = tok_by_expert[ea]
        toks[C : C + counts[eb]] = tok_by_expert[eb]
        core_tokens.append(toks)

        # x blocks in execution order; remainder groups packed compactly
        xg = np.zeros((n_groups, 128, 4 * GROUP), dtype=BF16)
        for i, (slot, off, n) in enumerate(groups):
            blk = x_bf[toks[slot * C + off : slot * C + off + n]]  # [n, 512]
            # [n, 4, 128] -> [128, 4, n] -> [128, 4*n]
            xg[i, :, : 4 * n] = (
                blk.reshape(n, 4, 128).transpose(2, 1, 0).reshape(128, 4 * n)
            )

        cb = np.zeros((128, cb_cols), dtype=BF16)
        w1p = np.zeros((2, HID, 128), dtype=np.float32)
        w1p[0, :, :EXP_HID] = W1[ea]
        w1p[1, :, :EXP_HID] = W1[eb]
        cb[:, CB_W1 : CB_W1 + 512] = (
            w1p.reshape(2, 2, 128, 128).transpose(2, 0, 1, 3).reshape(128, 512)
        ).astype(BF16)
        b1p = np.ones((128, 2), dtype=np.float32)
        b1p[:EXP_HID] = b1[[ea, eb]].T
        cb[:, CB_B1 : CB_B1 + 4] = np.ascontiguousarray(b1p).view(BF16)
        w2v = np.zeros((128, 2), dtype=np.float32)
        w2v[:EXP_HID] = W2[[ea, eb], :, 0].T
        w2v[EXP_HID] = b2[[ea, eb], 0]
        for i, (slot, off, n) in enumerate(groups):
            lane = i % SB
            cb[:, CB_W2 + 128 * i + lane] = w2v[:, slot].astype(BF16)

        in_maps.append({"xg": xg, "ca": ca, "cb": cb})

    res = run_bass_kernel_spmd(
        nc,
        in_maps,
        core_ids=list(range(N_CORES)),
        trace=_trace,
        **({"trace_cores": [0]} if _trace else {}),
    )
    if _result_box is not None:
        _result_box.append(res)

    out = np.zeros((B, OUT_DIM), dtype=np.float32)
    for c in range(N_CORES):
        oc = res.results[c]["out"]  # [SB, n_sb, GROUP] f32
        toks = core_tokens[c]
        for i, (slot, off, n) in enumerate(groups):
            sb, lane = divmod(i, SB)
            # skip padding rows (they alias token 0)
            v = max(0, min(n, int(counts[2 * c + slot]) - off))
            if v:
                out[toks[slot * C + off : slot * C + off + v], 0] = oc[lane, sb, :v]
    return out


# revision 17
# speedup vs baseline: 1.0568x; 1.0064x over previous
"""MoE routing kernel for 8 Trainium2 NeuronCores.

Problem: B=65536 tokens, shared Linear(512->256)+ReLU, then per-token expert
MLP Linear(256->100)+ReLU -> Linear(100->1), expert chosen by idx in [0,16).

Strategy (expert-parallel, host-side routing):
  - Host sorts tokens by expert. Experts 2c and 2c+1 go to core c, each in a
    fixed-capacity slot of C tokens (C = max expert count rounded up to 128),
    padded with token 0 (padding outputs are computed then discarded).
  - Host pre-transposes x to [512, TOK] bf16 per core so the contraction dim
    (IN_DIM) lands on SBUF partitions: all three GEMMs then chain on-chip with
    no transposes (layer-1 out [hid, tok] feeds layer-2, which feeds layer-3).
  - Device: per group of <=512 tokens: one DMA of x columns, 8 accumulating
    matmuls (512-dim contraction, 2 hid chunks) + bias/ReLU, 2 matmuls for
    expert FC1 + bias/ReLU, 1 matmul for FC2 (bias folded in via a ones-row).
  - All weights/biases ride in TWO packed contiguous bf16 DMAs on the ACT
    HWDGE ring; the SP ring carries only the x stream, so the first x group
    lands as early as possible.
  - Layer-3 outputs for 8 consecutive groups accumulate into one [8, 512]
    PSUM tile (per-group stationary is [128, 8] with a single nonzero column),
    so the store path is 3 copies + 3 multi-partition DMAs instead of a slow
    single-partition tail.
"""

import math
import os
import sys

import numpy as np

for _p in ("/opt/trn_rl_repo", "/opt/pypackages"):
    if _p not in sys.path and os.path.isdir(_p):
        sys.path.append(_p)

import ml_dtypes

BF16 = ml_dtypes.bfloat16

B, IN_DIM, HID, EXP_HID, OUT_DIM, N_EXP = 65536, 512, 256, 100, 1, 16
N_CORES = 8
GROUP = 512  # tokens per matmul group (= PSUM bank free-dim in fp32)
SB = 8  # groups per output super-block

# packed const layouts (bf16 columns per partition); biases are f32 stored
# as raw bytes (2 bf16 cols per value) and bitcast on device.
# constA is hc-major and split into two DMAs so the first L1 matmuls only
# gate on the first half: [ws hc0 512 | bs 4 | ws hc1 512]
CA_BS = 512
CA_COLS = 1028
CB_W1, CB_B1, CB_W2 = 0, 512, 516  # constB: w1 [2*2*128], b1 [2 f32], w2sel
# w2sel is one 128-wide block per group (only column `lane` nonzero) so its
# LDWEIGHTS is FWL-eligible and hides behind the preceding matmul


def _ca_col(hc, kc):
    return hc * 516 + kc * 128

_PROGRAM_CACHE = {}


def _make_groups(CA, CB):
    """Execution-ordered blocks: (slot, off, n); slot capacities (CA, CB).

    Remainder (short) blocks run last: they form the final output super-block,
    so the last L3 -> copy -> store chain is short.
    """
    full, rem = [], []
    for slot, cap in ((0, CA), (1, CB)):
        off = 0
        while off < cap:
            n = min(GROUP, cap - off)
            (full if n == GROUP else rem).append((slot, off, n))
            off += n
    return full + rem


def _build_program(CA: int, CB: int):
    """Build (and cache) the Bass program for slot capacities (CA, CB)."""
    import concourse.bass as bass
    import concourse.mybir as mybir
    import concourse.tile as tile
    from concourse import bacc

    TOK = CA + CB
    f32 = mybir.dt.float32
    bf16 = mybir.dt.bfloat16
    AF = mybir.ActivationFunctionType
    ALU = mybir.AluOpType

    groups = _make_groups(CA, CB)
    n_groups = len(groups)
    n_sb = (n_groups + SB - 1) // SB
    cb_cols = CB_W2 + 128 * n_groups

    nc = bacc.Bacc("TRN2", target_bir_lowering=False, debug=False)

    # xg[i] holds the i-th executed group's x block: for full groups
    # xg[i, p, kc*512 + t] = x[tok, kc*128 + p]; remainder groups are packed
    # compactly as xg[i, p, kc*n + t] (contiguous HBM reads).
    xg_d = nc.dram_tensor(
        "xg", [n_groups, 128, 4 * GROUP], bf16, kind="ExternalInput"
    ).ap()
    ca_d = nc.dram_tensor("ca", [128, CA_COLS], bf16, kind="ExternalInput").ap()
    cb_d = nc.dram_tensor("cb", [128, cb_cols], bf16, kind="ExternalInput").ap()
    out_d = nc.dram_tensor("out", [SB, n_sb, GROUP], f32, kind="ExternalOutput").ap()

    with tile.TileContext(nc) as tc:
        with (
            tc.tile_pool(name="const", bufs=1) as const,
            tc.tile_pool(name="xp", bufs=8) as xp,
            tc.tile_pool(name="hp", bufs=5) as hp,
            tc.tile_pool(name="h1p", bufs=5) as h1p,
            tc.tile_pool(name="ps1", bufs=5, space="PSUM") as ps1,
            tc.tile_pool(name="ps2", bufs=1, space="PSUM") as ps2,
            tc.tile_pool(name="ps3", bufs=2, space="PSUM") as ps3,
        ):
            ca_sb = const.tile([128, CA_COLS], bf16)
            cb_sb = const.tile([128, cb_cols], bf16)
            o2 = const.tile([SB, n_sb, GROUP], f32)
            bs_f32 = ca_sb[:, CA_BS : CA_BS + 4].bitcast(f32)  # [128, 2]
            b1_f32 = cb_sb[:, CB_B1 : CB_B1 + 4].bitcast(f32)  # [128, 2]

            # ca rides the ACT ring (hc0 weights + biases land first); cb is
            # issued on the SP ring *after* x1 so it does not steal bandwidth
            # from the startup-critical x transfers (ring order is FIFO).
            nc.scalar.dma_start(ca_sb[:, :516], ca_d[:, :516])
            nc.scalar.dma_start(ca_sb[:, 516:], ca_d[:, 516:])

            x_tiles = []
            for i, (_, _, n) in enumerate(groups):
                x_sb = xp.tile([128, 4, GROUP], bf16, tag="x", name=f"x_sb{i}")
                if n == GROUP:
                    nc.sync.dma_start(x_sb.rearrange("p c t -> p (c t)"), xg_d[i])
                else:
                    nc.sync.dma_start(
                        x_sb[:, :, :n],
                        xg_d[i, :, : 4 * n].rearrange("p (c t) -> p c t", c=4),
                    )
                x_tiles.append(x_sb)
                if i == 1:
                    nc.sync.dma_start(cb_sb[:, :CB_W2], cb_d[:, :CB_W2])
                if i == 3:
                    nc.sync.dma_start(cb_sb[:, CB_W2:], cb_d[:, CB_W2:])

            def xin(i, kc, n):
                return x_tiles[i][:, kc, :n]

            # PE warm-up while the first x DMA is in flight: keeps the HAM
            # activity window busy from preamble-end so the clock gate opens
            # (2.4 GHz) early in the real matmul stream.
            warm_w = const.tile([128, 256], bf16)
            nc.vector.memset(warm_w[:, :], 0.0)
            warm_p = ps1.tile([128, GROUP], f32, tag="p1", name="warm_p")
            for _ in range(14):
                nc.tensor.matmul(
                    warm_p[:, :256], warm_w[:, :128], warm_w[:, :], start=True, stop=True
                )

            # Software-pipelined emission: the PE queue is in-order, so L2(g)
            # waiting on ScalarE's h(g) and L3(g) waiting on VectorE's h1(g)
            # would stall the whole stream. Emitting L1(g) | L2(g-2) | L3(g-4)
            # gives every activation two full groups of matmuls to hide
            # behind, absorbing cross-engine semaphore latency too.
            h_tiles = [None] * n_groups
            h1_tiles = [None] * n_groups
            p3_tiles = [None] * n_sb

            def emit_l1(i):
                _, _, n = groups[i]
                h_sb = hp.tile([128, 2, GROUP], bf16, tag="h", name=f"h_sb{i}")
                h_tiles[i] = h_sb
                for hc in range(2):
                    p1 = ps1.tile([128, GROUP], f32, tag="p1", name=f"p1_{i}_{hc}")
                    for kc in range(4):
                        nc.tensor.matmul(
                            p1[:, :n],
                            ca_sb[:, _ca_col(hc, kc) :][:, :128],
                            xin(i, kc, n),
                            start=(kc == 0),
                            stop=(kc == 3),
                        )
                    # h = relu(psum + bs): hc0 on VectorE, hc1 on ScalarE
                    if hc == 0:
                        nc.vector.tensor_scalar(
                            h_sb[:, hc, :n],
                            p1[:, :n],
                            bs_f32[:, hc : hc + 1],
                            0.0,
                            ALU.add,
                            ALU.max,
                        )
                    else:
                        nc.scalar.activation(
                            h_sb[:, hc, :n],
                            p1[:, :n],
                            AF.Relu,
                            bias=bs_f32[:, hc : hc + 1],
                        )

            def emit_l2(i):
                e, _, n = groups[i]
                h_sb = h_tiles[i]
                p2 = ps2.tile([128, GROUP], f32, tag="p2", name=f"p2_{i}")
                for kc in range(2):
                    nc.tensor.matmul(
                        p2[:, :n],
                        cb_sb[:, CB_W1 + e * 256 + kc * 128 :][:, :128],
                        h_sb[:, kc, :n],
                        start=(kc == 0),
                        stop=(kc == 1),
                    )
                # h1 rows 0..99 = relu(psum + b1); rows 100..127 = relu(0+1)=1
                h1_sb = h1p.tile([128, GROUP], bf16, tag="h1", name=f"h1_sb{i}")
                h1_tiles[i] = h1_sb
                nc.vector.tensor_scalar(
                    h1_sb[:, :n],
                    p2[:, :n],
                    b1_f32[:, e : e + 1],
                    0.0,
                    ALU.add,
                    ALU.max,
                )

            def emit_l3(i):
                _, _, n = groups[i]
                sb, lane = divmod(i, SB)
                # one [128, 8] stationary per group, only column `lane`
                # nonzero -> psum row `lane` gets this group's outputs
                if lane == 0:
                    p3_tiles[sb] = ps3.tile([128, GROUP], f32, tag="p3", name=f"p3_{sb}")
                p3t = p3_tiles[sb]
                last_in_sb = (i == n_groups - 1) or (lane == SB - 1)
                nc.tensor.matmul(
                    p3t[:, :n],
                    cb_sb[:, CB_W2 + 128 * i : CB_W2 + 128 * i + 128],
                    h1_tiles[i][:, :n],
                    start=(lane == 0),
                    stop=last_in_sb,
                )
                if last_in_sb:
                    w = max(
                        groups[j][2]
                        for j in range(sb * SB, min((sb + 1) * SB, n_groups))
                    )
                    if sb == n_sb - 1:
                        # final block: DVE copy + SP-ring store, so it does
                        # not serialize behind the previous block's ACT-ring
                        # copy and DMA issue
                        nc.vector.tensor_scalar_add(o2[:, sb, :w], p3t[:8, :w], 0.0)
                        nc.sync.dma_start(out_d[:, sb, :w], o2[:, sb, :w])
                    else:
                        nc.scalar.copy(o2[:, sb, :w], p3t[:8, :w])
                        nc.scalar.dma_start(out_d[:, sb, :w], o2[:, sb, :w])

            for g in range(n_groups):
                emit_l1(g)
                if g >= 2:
                    emit_l2(g - 2)
                if g >= 4:
                    emit_l3(g - 4)
            # drain: remaining L2s first so their h1 activations are already
            # in flight when the trailing L3s (nothing left to hide behind)
            # come to wait on them
            for i in range(max(0, n_groups - 2), n_groups):
                emit_l2(i)
            for i in range(max(0, n_groups - 4), n_groups):
                emit_l3(i)

    nc.compile()
    return nc


def _get_program(CA: int, CB: int):
    if (CA, CB) not in _PROGRAM_CACHE:
        _PROGRAM_CACHE[(CA, CB)] = _build_program(CA, CB)
    return _PROGRAM_CACHE[(CA, CB)]


def kernel(x, idx, Ws, bs, W1, b1, W2, b2, _trace=False, _result_box=None):
    from concourse.bass_utils import run_bass_kernel_spmd

    x = np.asarray(x)
    idx = np.asarray(idx).astype(np.int64)
    Ws = np.asarray(Ws, dtype=np.float32)
    bs = np.asarray(bs, dtype=np.float32)
    W1 = np.asarray(W1, dtype=np.float32)
    b1 = np.asarray(b1, dtype=np.float32)
    W2 = np.asarray(W2, dtype=np.float32)
    b2 = np.asarray(b2, dtype=np.float32)

    counts = np.bincount(idx, minlength=N_EXP)
    # pair the 8 largest experts with the 8 smallest: slot-A capacity covers
    # the global max, slot-B only the max of the small half, cutting padding
    srt = np.argsort(-counts, kind="stable")
    big, small = srt[:8], srt[8:]
    CA = max(GROUP, int(math.ceil(counts[big].max() / 128) * 128))
    CB = max(GROUP, int(math.ceil(counts[small].max() / 128) * 128))
    nc = _get_program(CA, CB)

    groups = _make_groups(CA, CB)
    n_groups = len(groups)
    n_sb = (n_groups + SB - 1) // SB
    cb_cols = CB_W2 + 128 * n_groups

    order = np.argsort(idx, kind="stable")
    bounds = np.zeros(N_EXP + 1, dtype=np.int64)
    np.cumsum(counts, out=bounds[1:])
    tok_by_expert = [order[bounds[e] : bounds[e + 1]] for e in range(N_EXP)]

    # constA (same for every core): ws blocked so [:, kc*256+hc*128:+128] is
    # the (kc, hc) stationary; bs as [128, 2] columns
    ca = np.zeros((128, CA_COLS), dtype=BF16)
    wsb = Ws.reshape(4, 128, 2, 128).astype(BF16)  # [kc, p, hc, j]
    for hc in range(2):
        for kc in range(4):
            ca[:, _ca_col(hc, kc) : _ca_col(hc, kc) + 128] = wsb[:, :, hc][kc]
    ca[:, CA_BS : CA_BS + 4] = np.ascontiguousarray(
        bs.reshape(2, 128).T.astype(np.float32)
    ).view(BF16)

    x_bf = x.astype(BF16)
    in_maps = []
    core_tokens = []
    for c in range(N_CORES):
        ea, eb = int(big[c]), int(small[c])
        toks = np.zeros(CA + CB, dtype=np.int64)
        toks[: counts[ea]] = tok_by_expert[ea]
        toks[CA : CA + counts[eb]] = tok_by_expert[eb]
        core_tokens.append(toks)

        # x blocks in execution order; remainder groups packed compactly
        xg = np.zeros((n_groups, 128, 4 * GROUP), dtype=BF16)
        for i, (slot, off, n) in enumerate(groups):
            base = slot * CA
            blk = x_bf[toks[base + off : base + off + n]]  # [n, 512]
            # [n, 4, 128] -> [128, 4, n] -> [128, 4*n]
            xg[i, :, : 4 * n] = (
                blk.reshape(n, 4, 128).transpose(2, 1, 0).reshape(128, 4 * n)
            )

        cb = np.zeros((128, cb_cols), dtype=BF16)
        w1p = np.zeros((2, HID, 128), dtype=np.float32)
        w1p[0, :, :EXP_HID] = W1[ea]
        w1p[1, :, :EXP_HID] = W1[eb]
        cb[:, CB_W1 : CB_W1 + 512] = (
            w1p.reshape(2, 2, 128, 128).transpose(2, 0, 1, 3).reshape(128, 512)
        ).astype(BF16)
        b1p = np.ones((128, 2), dtype=np.float32)
        b1p[:EXP_HID] = b1[[ea, eb]].T
        cb[:, CB_B1 : CB_B1 + 4] = np.ascontiguousarray(b1p).view(BF16)
        w2v = np.zeros((128, 2), dtype=np.float32)
        w2v[:EXP_HID] = W2[[ea, eb], :, 0].T
        w2v[EXP_HID] = b2[[ea, eb], 0]
        for i, (slot, off, n) in enumerate(groups):
            lane = i % SB
            cb[:, CB_W2 + 128 * i + lane] = w2v[:, slot].astype(BF16)

        in_maps.append({"xg": xg, "ca": ca, "cb": cb})

    res = run_bass_kernel_spmd(
        nc,
        in_maps,
        core_ids=list(range(N_CORES)),
        trace=_trace,
        **({"trace_cores": [0]} if _trace else {}),
    )
    if _result_box is not None:
        _result_box.append(res)

    out = np.zeros((B, OUT_DIM), dtype=np.float32)
    for c in range(N_CORES):
        oc = res.results[c]["out"]  # [SB, n_sb, GROUP] f32
        toks = core_tokens[c]
        pair = (int(big[c]), int(small[c]))
        for i, (slot, off, n) in enumerate(groups):
            sb, lane = divmod(i, SB)
            base = slot * CA
            # skip padding rows (they alias token 0)
            v = max(0, min(n, int(counts[pair[slot]]) - off))
            if v:
                out[toks[base + off : base + off + v], 0] = oc[lane, sb, :v]
    return out


# revision 18
# speedup vs baseline: 1.0666x; 1.0093x over previous
"""MoE routing kernel for 8 Trainium2 NeuronCores.

Problem: B=65536 tokens, shared Linear(512->256)+ReLU, then per-token expert
MLP Linear(256->100)+ReLU -> Linear(100->1), expert chosen by idx in [0,16).

Strategy (expert-parallel, host-side routing):
  - Host sorts tokens by expert. Experts 2c and 2c+1 go to core c, each in a
    fixed-capacity slot of C tokens (C = max expert count rounded up to 128),
    padded with token 0 (padding outputs are computed then discarded).
  - Host pre-transposes x to [512, TOK] bf16 per core so the contraction dim
    (IN_DIM) lands on SBUF partitions: all three GEMMs then chain on-chip with
    no transposes (layer-1 out [hid, tok] feeds layer-2, which feeds layer-3).
  - Device: per group of <=512 tokens: one DMA of x columns, 8 accumulating
    matmuls (512-dim contraction, 2 hid chunks) + bias/ReLU, 2 matmuls for
    expert FC1 + bias/ReLU, 1 matmul for FC2 (bias folded in via a ones-row).
  - All weights/biases ride in TWO packed contiguous bf16 DMAs on the ACT
    HWDGE ring; the SP ring carries only the x stream, so the first x group
    lands as early as possible.
  - Layer-3 outputs for 8 consecutive groups accumulate into one [8, 512]
    PSUM tile (per-group stationary is [128, 8] with a single nonzero column),
    so the store path is 3 copies + 3 multi-partition DMAs instead of a slow
    single-partition tail.
"""

import math
import os
import sys

import numpy as np

for _p in ("/opt/trn_rl_repo", "/opt/pypackages"):
    if _p not in sys.path and os.path.isdir(_p):
        sys.path.append(_p)

import ml_dtypes

BF16 = ml_dtypes.bfloat16

B, IN_DIM, HID, EXP_HID, OUT_DIM, N_EXP = 65536, 512, 256, 100, 1, 16
N_CORES = 8
GROUP = 512  # tokens per matmul group (= PSUM bank free-dim in fp32)
SB = 8  # groups per output super-block

# packed const layouts (bf16 columns per partition); biases are f32 stored
# as raw bytes (2 bf16 cols per value) and bitcast on device.
# constA is hc-major and split into two DMAs so the first L1 matmuls only
# gate on the first half: [ws hc0 512 | bs 4 | ws hc1 512]
CA_BS = 512
CA_COLS = 1028
CB_W1, CB_B1, CB_W2 = 0, 512, 516  # constB: w1 [2*2*128], b1 [2 f32], w2sel
# w2sel is one 128-wide block per group (only column `lane` nonzero) so its
# LDWEIGHTS is FWL-eligible and hides behind the preceding matmul


def _ca_col(hc, kc):
    return hc * 516 + kc * 128

_PROGRAM_CACHE = {}


def _make_groups(CA, CB):
    """Execution-ordered blocks: (slot, off, n); slot capacities (CA, CB).

    Remainder (short) blocks run last: they form the final output super-block,
    so the last L3 -> copy -> store chain is short.
    """
    full, rem = [], []
    for slot, cap in ((0, CA), (1, CB)):
        off = 0
        while off < cap:
            n = min(GROUP, cap - off)
            (full if n == GROUP else rem).append((slot, off, n))
            off += n
    return full + rem


def _build_program(CA: int, CB: int):
    """Build (and cache) the Bass program for slot capacities (CA, CB)."""
    import concourse.bass as bass
    import concourse.mybir as mybir
    import concourse.tile as tile
    from concourse import bacc

    TOK = CA + CB
    f32 = mybir.dt.float32
    bf16 = mybir.dt.bfloat16
    AF = mybir.ActivationFunctionType
    ALU = mybir.AluOpType

    groups = _make_groups(CA, CB)
    n_groups = len(groups)
    n_sb = (n_groups + SB - 1) // SB
    cb_cols = CB_W2 + 128 * n_groups

    nc = bacc.Bacc("TRN2", target_bir_lowering=False, debug=False)

    # xg[i] holds the i-th executed group's x block: for full groups
    # xg[i, p, kc*512 + t] = x[tok, kc*128 + p]; remainder groups are packed
    # compactly as xg[i, p, kc*n + t] (contiguous HBM reads).
    xg_d = nc.dram_tensor(
        "xg", [n_groups, 128, 4 * GROUP], bf16, kind="ExternalInput"
    ).ap()
    ca_d = nc.dram_tensor("ca", [128, CA_COLS], bf16, kind="ExternalInput").ap()
    cb_d = nc.dram_tensor("cb", [128, cb_cols], bf16, kind="ExternalInput").ap()
    out_d = nc.dram_tensor("out", [SB, n_sb, GROUP], f32, kind="ExternalOutput").ap()

    with tile.TileContext(nc) as tc:
        with (
            tc.tile_pool(name="const", bufs=1) as const,
            tc.tile_pool(name="xp", bufs=8) as xp,
            tc.tile_pool(name="hp", bufs=5) as hp,
            tc.tile_pool(name="h1p", bufs=5) as h1p,
            tc.tile_pool(name="ps1", bufs=5, space="PSUM") as ps1,
            tc.tile_pool(name="ps2", bufs=1, space="PSUM") as ps2,
            tc.tile_pool(name="ps3", bufs=2, space="PSUM") as ps3,
        ):
            ca_sb = const.tile([128, CA_COLS], bf16)
            cb_sb = const.tile([128, cb_cols], bf16)
            o2 = const.tile([SB, n_sb, GROUP], f32)
            bs_f32 = ca_sb[:, CA_BS : CA_BS + 4].bitcast(f32)  # [128, 2]
            b1_f32 = cb_sb[:, CB_B1 : CB_B1 + 4].bitcast(f32)  # [128, 2]

            # ca rides the ACT ring (hc0 weights + biases land first); cb is
            # issued on the SP ring *after* x1 so it does not steal bandwidth
            # from the startup-critical x transfers (ring order is FIFO).
            nc.scalar.dma_start(ca_sb[:, :516], ca_d[:, :516])
            nc.scalar.dma_start(ca_sb[:, 516:], ca_d[:, 516:])

            # each group's x comes as two half DMAs (kc 0-1, kc 2-3): with
            # subtile dependency tracking the first L1 matmuls only gate on
            # the first half, smoothing the DMA-fed ramp at startup
            x_tiles = []
            for i, (_, _, n) in enumerate(groups):
                x_sb = xp.tile([128, 4, GROUP], bf16, tag="x", name=f"x_sb{i}")
                if n == GROUP:
                    for hf in range(2):
                        nc.sync.dma_start(
                            x_sb[:, 2 * hf : 2 * hf + 2, :].rearrange(
                                "p c t -> p (c t)"
                            ),
                            xg_d[i, :, 2 * hf * GROUP : (2 * hf + 2) * GROUP],
                        )
                else:
                    nc.sync.dma_start(
                        x_sb[:, :, :n],
                        xg_d[i, :, : 4 * n].rearrange("p (c t) -> p c t", c=4),
                    )
                x_tiles.append(x_sb)
                if i == 1:
                    nc.sync.dma_start(cb_sb[:, :CB_W2], cb_d[:, :CB_W2])
                if i == 3:
                    nc.sync.dma_start(cb_sb[:, CB_W2:], cb_d[:, CB_W2:])

            def xin(i, kc, n):
                return x_tiles[i][:, kc, :n]

            # PE warm-up while the first x DMA is in flight: keeps the HAM
            # activity window busy from preamble-end so the clock gate opens
            # (2.4 GHz) early in the real matmul stream.
            warm_w = const.tile([128, 256], bf16)
            nc.vector.memset(warm_w[:, :], 0.0)
            warm_p = ps1.tile([128, GROUP], f32, tag="p1", name="warm_p")
            for _ in range(14):
                nc.tensor.matmul(
                    warm_p[:, :256], warm_w[:, :128], warm_w[:, :], start=True, stop=True
                )

            # Software-pipelined emission: the PE queue is in-order, so L2(g)
            # waiting on ScalarE's h(g) and L3(g) waiting on VectorE's h1(g)
            # would stall the whole stream. Emitting L1(g) | L2(g-2) | L3(g-4)
            # gives every activation two full groups of matmuls to hide
            # behind, absorbing cross-engine semaphore latency too.
            h_tiles = [None] * n_groups
            h1_tiles = [None] * n_groups
            p3_tiles = [None] * n_sb

            def emit_l1(i):
                _, _, n = groups[i]
                h_sb = hp.tile([128, 2, GROUP], bf16, tag="h", name=f"h_sb{i}")
                h_tiles[i] = h_sb
                for hc in range(2):
                    p1 = ps1.tile([128, GROUP], f32, tag="p1", name=f"p1_{i}_{hc}")
                    for kc in range(4):
                        nc.tensor.matmul(
                            p1[:, :n],
                            ca_sb[:, _ca_col(hc, kc) :][:, :128],
                            xin(i, kc, n),
                            start=(kc == 0),
                            stop=(kc == 3),
                        )
                    # h = relu(psum + bs): hc0 on VectorE, hc1 on ScalarE
                    if hc == 0:
                        nc.vector.tensor_scalar(
                            h_sb[:, hc, :n],
                            p1[:, :n],
                            bs_f32[:, hc : hc + 1],
                            0.0,
                            ALU.add,
                            ALU.max,
                        )
                    else:
                        nc.scalar.activation(
                            h_sb[:, hc, :n],
                            p1[:, :n],
                            AF.Relu,
                            bias=bs_f32[:, hc : hc + 1],
                        )

            def emit_l2(i):
                e, _, n = groups[i]
                h_sb = h_tiles[i]
                p2 = ps2.tile([128, GROUP], f32, tag="p2", name=f"p2_{i}")
                for kc in range(2):
                    nc.tensor.matmul(
                        p2[:, :n],
                        cb_sb[:, CB_W1 + e * 256 + kc * 128 :][:, :128],
                        h_sb[:, kc, :n],
                        start=(kc == 0),
                        stop=(kc == 1),
                    )
                # h1 rows 0..99 = relu(psum + b1); rows 100..127 = relu(0+1)=1
                h1_sb = h1p.tile([128, GROUP], bf16, tag="h1", name=f"h1_sb{i}")
                h1_tiles[i] = h1_sb
                nc.vector.tensor_scalar(
                    h1_sb[:, :n],
                    p2[:, :n],
                    b1_f32[:, e : e + 1],
                    0.0,
                    ALU.add,
                    ALU.max,
                )

            def emit_l3(i):
                _, _, n = groups[i]
                sb, lane = divmod(i, SB)
                # one [128, 8] stationary per group, only column `lane`
                # nonzero -> psum row `lane` gets this group's outputs
                if lane == 0:
                    p3_tiles[sb] = ps3.tile([128, GROUP], f32, tag="p3", name=f"p3_{sb}")
                p3t = p3_tiles[sb]
                last_in_sb = (i == n_groups - 1) or (lane == SB - 1)
                nc.tensor.matmul(
                    p3t[:, :n],
                    cb_sb[:, CB_W2 + 128 * i : CB_W2 + 128 * i + 128],
                    h1_tiles[i][:, :n],
                    start=(lane == 0),
                    stop=last_in_sb,
                )
                if last_in_sb:
                    w = max(
                        groups[j][2]
                        for j in range(sb * SB, min((sb + 1) * SB, n_groups))
                    )
                    if sb == n_sb - 1:
                        # final block: DVE copy + SP-ring store, so it does
                        # not serialize behind the previous block's ACT-ring
                        # copy and DMA issue
                        nc.vector.tensor_scalar_add(o2[:, sb, :w], p3t[:8, :w], 0.0)
                        nc.sync.dma_start(out_d[:, sb, :w], o2[:, sb, :w])
                    else:
                        nc.scalar.copy(o2[:, sb, :w], p3t[:8, :w])
                        nc.scalar.dma_start(out_d[:, sb, :w], o2[:, sb, :w])

            for g in range(n_groups):
                emit_l1(g)
                if g >= 2:
                    emit_l2(g - 2)
                if g >= 4:
                    emit_l3(g - 4)
            # drain: remaining L2s first so their h1 activations are already
            # in flight when the trailing L3s (nothing left to hide behind)
            # come to wait on them
            for i in range(max(0, n_groups - 2), n_groups):
                emit_l2(i)
            for i in range(max(0, n_groups - 4), n_groups):
                emit_l3(i)

    nc.compile()
    return nc


def _get_program(CA: int, CB: int):
    if (CA, CB) not in _PROGRAM_CACHE:
        _PROGRAM_CACHE[(CA, CB)] = _build_program(CA, CB)
    return _PROGRAM_CACHE[(CA, CB)]


def kernel(x, idx, Ws, bs, W1, b1, W2, b2, _trace=False, _result_box=None):
    from concourse.bass_utils import run_bass_kernel_spmd

    x = np.asarray(x)
    idx = np.asarray(idx).astype(np.int64)
    Ws = np.asarray(Ws, dtype=np.float32)
    bs = np.asarray(bs, dtype=np.float32)
    W1 = np.asarray(W1, dtype=np.float32)
    b1 = np.asarray(b1, dtype=np.float32)
    W2 = np.asarray(W2, dtype=np.float32)
    b2 = np.asarray(b2, dtype=np.float32)

    counts = np.bincount(idx, minlength=N_EXP)
    # pair the 8 largest experts with the 8 smallest: slot-A capacity covers
    # the global max, slot-B only the max of the small half, cutting padding
    srt = np.argsort(-counts, kind="stable")
    big, small = srt[:8], srt[8:]
    CA = max(GROUP, int(math.ceil(counts[big].max() / 128) * 128))
    CB = max(GROUP, int(math.ceil(counts[small].max() / 128) * 128))
    nc = _get_program(CA, CB)

    groups = _make_groups(CA, CB)
    n_groups = len(groups)
    n_sb = (n_groups + SB - 1) // SB
    cb_cols = CB_W2 + 128 * n_groups

    order = np.argsort(idx, kind="stable")
    bounds = np.zeros(N_EXP + 1, dtype=np.int64)
    np.cumsum(counts, out=bounds[1:])
    tok_by_expert = [order[bounds[e] : bounds[e + 1]] for e in range(N_EXP)]

    # constA (same for every core): ws blocked so [:, kc*256+hc*128:+128] is
    # the (kc, hc) stationary; bs as [128, 2] columns
    ca = np.zeros((128, CA_COLS), dtype=BF16)
    wsb = Ws.reshape(4, 128, 2, 128).astype(BF16)  # [kc, p, hc, j]
    for hc in range(2):
        for kc in range(4):
            ca[:, _ca_col(hc, kc) : _ca_col(hc, kc) + 128] = wsb[:, :, hc][kc]
    ca[:, CA_BS : CA_BS + 4] = np.ascontiguousarray(
        bs.reshape(2, 128).T.astype(np.float32)
    ).view(BF16)

    x_bf = x.astype(BF16)
    in_maps = []
    core_tokens = []
    for c in range(N_CORES):
        ea, eb = int(big[c]), int(small[c])
        toks = np.zeros(CA + CB, dtype=np.int64)
        toks[: counts[ea]] = tok_by_expert[ea]
        toks[CA : CA + counts[eb]] = tok_by_expert[eb]
        core_tokens.append(toks)

        # x blocks in execution order; remainder groups packed compactly
        xg = np.zeros((n_groups, 128, 4 * GROUP), dtype=BF16)
        for i, (slot, off, n) in enumerate(groups):
            base = slot * CA
            blk = x_bf[toks[base + off : base + off + n]]  # [n, 512]
            # [n, 4, 128] -> [128, 4, n] -> [128, 4*n]
            xg[i, :, : 4 * n] = (
                blk.reshape(n, 4, 128).transpose(2, 1, 0).reshape(128, 4 * n)
            )

        cb = np.zeros((128, cb_cols), dtype=BF16)
        w1p = np.zeros((2, HID, 128), dtype=np.float32)
        w1p[0, :, :EXP_HID] = W1[ea]
        w1p[1, :, :EXP_HID] = W1[eb]
        cb[:, CB_W1 : CB_W1 + 512] = (
            w1p.reshape(2, 2, 128, 128).transpose(2, 0, 1, 3).reshape(128, 512)
        ).astype(BF16)
        b1p = np.ones((128, 2), dtype=np.float32)
        b1p[:EXP_HID] = b1[[ea, eb]].T
        cb[:, CB_B1 : CB_B1 + 4] = np.ascontiguousarray(b1p).view(BF16)
        w2v = np.zeros((128, 2), dtype=np.float32)
        w2v[:EXP_HID] = W2[[ea, eb], :, 0].T
        w2v[EXP_HID] = b2[[ea, eb], 0]
        for i, (slot, off, n) in enumerate(groups):
            lane = i % SB
            cb[:, CB_W2 + 128 * i + lane] = w2v[:, slot].astype(BF16)

        in_maps.append({"xg": xg, "ca": ca, "cb": cb})

    res = run_bass_kernel_spmd(
        nc,
        in_maps,
        core_ids=list(range(N_CORES)),
        trace=_trace,
        **({"trace_cores": [0]} if _trace else {}),
    )
    if _result_box is not None:
        _result_box.append(res)

    out = np.zeros((B, OUT_DIM), dtype=np.float32)
    for c in range(N_CORES):
        oc = res.results[c]["out"]  # [SB, n_sb, GROUP] f32
        toks = core_tokens[c]
        pair = (int(big[c]), int(small[c]))
        for i, (slot, off, n) in enumerate(groups):
            sb, lane = divmod(i, SB)
            base = slot * CA
            # skip padding rows (they alias token 0)
            v = max(0, min(n, int(counts[pair[slot]]) - off))
            if v:
                out[toks[base + off : base + off + v], 0] = oc[lane, sb, :v]
    return out
